# revision 23
# baseline (speedup 1.0000x reference)
"""DA-RNN Trainium2 Bass kernel.

Data-parallel over batch: 256 batch / 8 cores = 32 per core.
Recurrence over S=512 steps runs locally per core, feature-major layout
([feature_partition, batch] tiles). Temporal attention is computed as
running sums during the scan (U = sum_t e_t * h_t, Z = sum_t e_t), so no
encoder buffer is materialized.

All gate/score biases are injected into PSUM by tiny K=1 matmuls on the
(mostly idle) PE, so each activation stage is a single wide ACT op.
sigmoid(x) is computed as 0.5 + 0.5*tanh(x/2) to stay inside the single
exp_and_others ACT table set (exp+tanh) — a set switch costs ~2.7us.
"""

import numpy as np

import concourse.bass as bass
import concourse.mybir as mybir
import concourse.tile as tile
from concourse import bacc
from concourse.bass_utils import run_bass_kernel_spmd

F32 = mybir.dt.float32
BF16 = mybir.dt.bfloat16
AF = mybir.ActivationFunctionType
ALU = mybir.AluOpType

B, S, I, H, O = 256, 512, 128, 256, 1
NCORES = 8
BL = B // NCORES  # 32 local batch

MM_DT = BF16  # dtype for heavy matmul operands (halves LDWEIGHTS via FWL)


def _build_program(n_steps: int):
    nc = bacc.Bacc(None, target_bir_lowering=False)

    # ---- DRAM I/O (per-core shapes; weights replicated across cores) ----
    x_d = nc.dram_tensor("x", [128, n_steps * BL], F32, kind="ExternalInput")
    wa_d = nc.dram_tensor("wa", [128, 3 * 128], F32, kind="ExternalInput")
    wih_d = nc.dram_tensor("wih", [128, 768], F32, kind="ExternalInput")
    whh_d = nc.dram_tensor("whh", [128, 2 * 768], F32, kind="ExternalInput")
    # bias values for PE injection (partition-major rows) + selectors
    brz4_d = nc.dram_tensor("brz4", [4, 128], F32, kind="ExternalInput")
    bin2_d = nc.dram_tensor("bin2", [2, 128], F32, kind="ExternalInput")
    bhn2_d = nc.dram_tensor("bhn2", [2, 128], F32, kind="ExternalInput")
    sel4_d = nc.dram_tensor("sel4", [4, 128], F32, kind="ExternalInput")
    sel2_d = nc.dram_tensor("sel2", [2, 64], F32, kind="ExternalInput")
    ba_d = nc.dram_tensor("ba", [128, 1], F32, kind="ExternalInput")
    wt_d = nc.dram_tensor("wt", [128, 2], F32, kind="ExternalInput")
    bt_d = nc.dram_tensor("bt", [1, 1], F32, kind="ExternalInput")
    wf_d = nc.dram_tensor("wf", [128, 2], F32, kind="ExternalInput")
    bf_d = nc.dram_tensor("bf", [1, 1], F32, kind="ExternalInput")  # 0.5*b_f
    out_d = nc.dram_tensor("out", [1, BL], F32, kind="ExternalOutput")

    with tile.TileContext(nc) as tc:
        with (
            tc.tile_pool(name="big", bufs=1) as big,
            tc.tile_pool(name="wpool", bufs=1) as wpool,
            tc.tile_pool(name="state", bufs=1) as state,
            tc.tile_pool(name="hpool", bufs=2) as hpool,
            tc.tile_pool(name="work", bufs=2) as work,
            tc.tile_pool(name="ps_sc", bufs=1, space="PSUM") as ps_sc_pool,
            tc.tile_pool(name="ps_rz", bufs=1, space="PSUM") as ps_rz_pool,
            tc.tile_pool(name="ps_hn", bufs=1, space="PSUM") as ps_hn_pool,
            tc.tile_pool(name="ps_in", bufs=1, space="PSUM") as ps_in_pool,
            tc.tile_pool(name="ps_sum", bufs=1, space="PSUM") as ps_sum_pool,
            tc.tile_pool(name="ps_bc", bufs=1, space="PSUM") as ps_bc_pool,
            tc.tile_pool(name="ps_ts", bufs=1, space="PSUM") as ps_ts_pool,
            tc.tile_pool(name="ps_ebc", bufs=1, space="PSUM") as ps_ebc_pool,
        ):
            # ACT table warmup: first activation triggers the walrus-inserted
            # table load (needs its own syncs) — keep it dependency-light.
            warm = state.tile([1, 2], F32, tag="warm")
            nc.gpsimd.memset(warm[:], 0.0)
            nc.scalar.activation(warm[:], warm[:], AF.Tanh)
            nc.scalar.activation(warm[:], warm[:], AF.Exp)

            # ---- load inputs into SBUF ----
            x_sb = big.tile([128, n_steps * BL], F32)
            nchunk = 8
            cw = (n_steps * BL) // nchunk
            for c in range(nchunk):
                nc.sync.dma_start(x_sb[:, c * cw:(c + 1) * cw], x_d[:, c * cw:(c + 1) * cw])

            def load_w(dram, shape, name):
                t = wpool.tile(shape, F32, tag=name)
                nc.sync.dma_start(t[:], dram[:])
                return t

            wa = load_w(wa_d, [128, 3 * 128], "wa")
            wih = load_w(wih_d, [128, 768], "wih")
            whh = load_w(whh_d, [128, 2 * 768], "whh")
            brz4 = load_w(brz4_d, [4, 128], "brz4")
            bin2 = load_w(bin2_d, [2, 128], "bin2")
            bhn2 = load_w(bhn2_d, [2, 128], "bhn2")
            sel4 = load_w(sel4_d, [4, 128], "sel4")
            sel2 = load_w(sel2_d, [2, 64], "sel2")
            ba = load_w(ba_d, [128, 1], "ba")
            wt = load_w(wt_d, [128, 2], "wt")
            bt = load_w(bt_d, [1, 1], "bt")
            wf = load_w(wf_d, [128, 2], "wf")
            bf = load_w(bf_d, [1, 1], "bf")

            x_mm = big.tile([128, n_steps * BL], MM_DT)
            for c in range(nchunk):
                nc.vector.tensor_copy(x_mm[:, c * cw:(c + 1) * cw],
                                      x_sb[:, c * cw:(c + 1) * cw])
            wa_m = wpool.tile([128, 3 * 128], MM_DT, tag="wa_m")
            nc.vector.tensor_copy(wa_m[:], wa[:])
            wih_m = wpool.tile([128, 768], MM_DT, tag="wih_m")
            nc.vector.tensor_copy(wih_m[:], wih[:])
            whh_m = wpool.tile([128, 2 * 768], MM_DT, tag="whh_m")
            nc.vector.tensor_copy(whh_m[:], whh[:])
            wt_m = wpool.tile([128, 2], MM_DT, tag="wt_m")
            nc.vector.tensor_copy(wt_m[:], wt[:])
            def to_mm(t, shape, name):
                tm = wpool.tile(shape, MM_DT, tag=name)
                nc.vector.tensor_copy(tm[:], t[:])
                return tm
            brz4_m = to_mm(brz4, [4, 128], "brz4_m")
            bin2_m = to_mm(bin2, [2, 128], "bin2_m")
            bhn2_m = to_mm(bhn2, [2, 128], "bhn2_m")
            sel4_m = to_mm(sel4, [4, 128], "sel4_m")
            sel2_m = to_mm(sel2, [2, 64], "sel2_m")

            ones_col = state.tile([128, 1], MM_DT, tag="ones_col")
            nc.vector.memset(ones_col[:], 1.0)
            ones_row = state.tile([1, 128], MM_DT, tag="ones_row")
            nc.vector.memset(ones_row[:], 1.0)
            ones_row_f = state.tile([1, 128], F32, tag="ones_row_f")
            nc.vector.memset(ones_row_f[:], 1.0)

            # ---- persistent state ----
            U = state.tile([128, 2 * BL], F32, tag="U")
            nc.vector.memset(U[:], 0.0)
            Z = state.tile([1, BL], F32, tag="Z")
            nc.vector.memset(Z[:], 0.0)
            h0_t = hpool.tile([128, 2 * BL], F32, tag="h")
            nc.vector.memset(h0_t[:], 0.0)
            hm0 = hpool.tile([128, 2 * BL], MM_DT, tag="hm")
            nc.vector.memset(hm0[:], 0.0)

            # ---- persistent PSUM tiles (8 banks total) ----
            ps_sc = ps_sc_pool.tile([128, BL], F32)
            ps_rz = ps_rz_pool.tile([128, 4 * BL], F32)
            ps_hn = ps_hn_pool.tile([128, 2 * BL], F32)
            ps_in = ps_in_pool.tile([128, 2 * BL], F32)
            ps_sum = ps_sum_pool.tile([1, BL], F32)
            ps_bc = ps_bc_pool.tile([128, BL], F32)
            ps_ts = ps_ts_pool.tile([1, BL], F32)
            ps_ebc = ps_ebc_pool.tile([128, 2 * BL], F32)

            h = h0_t   # fp32 h (state after previous step)
            hm = hm0   # matmul-dtype h

            MM = nc.tensor.matmul
            ones32 = ones_row[:, 0:BL]

            for t in range(n_steps):
                xt = x_mm[:, t * BL:(t + 1) * BL]

                # -- PE: scores = W_a @ [x_t; h]  (feature-major out [128, BL])
                MM(ps_sc[:], wa_m[:, 0:128], xt, start=True, stop=False)
                MM(ps_sc[:], wa_m[:, 128:256], hm[:, 0:BL], start=False, stop=False)
                MM(ps_sc[:], wa_m[:, 256:384], hm[:, BL:2 * BL], start=False, stop=True)

                # -- U-path part 1: t_score MMs on h (= enc state s_{t-1})
                if t > 0:
                    MM(ps_ts[:], wt_m[:, 0:1], hm[:, 0:BL], start=True, stop=False)
                    MM(ps_ts[:], wt_m[:, 1:2], hm[:, BL:2 * BL], start=False, stop=True)

                # -- PE: bias injections (no data deps — fill PE idle time)
                MM(ps_rz[:], brz4_m[:], sel4_m[:], start=True, stop=False)
                MM(ps_in[:], bin2_m[:], sel2_m[:], start=True, stop=False)
                MM(ps_hn[:], bhn2_m[:], sel2_m[:], start=True, stop=False)

                # -- PE: gh for r,z (accumulate; gi joins later) and for n
                for m in range(4):
                    for k in range(2):
                        MM(ps_rz[:, m * BL:(m + 1) * BL],
                           whh_m[:, k * 768 + m * 128:k * 768 + (m + 1) * 128],
                           hm[:, k * BL:(k + 1) * BL],
                           start=False, stop=False)
                for m in range(2):
                    for k in range(2):
                        MM(ps_hn[:, m * BL:(m + 1) * BL],
                           whh_m[:, k * 768 + (4 + m) * 128:k * 768 + (5 + m) * 128],
                           hm[:, k * BL:(k + 1) * BL],
                           start=False, stop=(k == 1))

                # -- ACT: exp(tanh(scores + b_a))
                tanh_sb = work.tile([128, BL], F32, tag="tanh_sb")
                nc.scalar.activation(tanh_sb[:], ps_sc[:], AF.Tanh, bias=ba[:, 0:1])
                exp_sb = work.tile([128, BL], MM_DT, tag="exp_sb")
                nc.scalar.activation(exp_sb[:], tanh_sb[:], AF.Exp)

                # -- U-path: e = exp(tanh(ts + b_t))
                if t > 0:
                    ts_sb = work.tile([1, BL], F32, tag="ts_sb")
                    nc.scalar.activation(ts_sb[:], ps_ts[:], AF.Tanh, bias=bt[:, 0:1])
                    e_f32 = work.tile([1, BL], F32, tag="e_f32")
                    nc.scalar.activation(e_f32[:], ts_sb[:], AF.Exp)
                else:
                    e_f32 = None

                # -- PE: softmax denominator (partition sum), then VEC recip
                MM(ps_sum[:], ones_col[:], exp_sb[:], start=True, stop=True)
                rsum = work.tile([1, BL], F32, tag="rsum")
                nc.vector.reciprocal(rsum[:], ps_sum[:])

                # -- U-path: broadcast e over partitions
                if e_f32 is not None:
                    MM(ps_ebc[:, 0:BL], ones_row_f[:], e_f32[:], start=True, stop=True)
                    MM(ps_ebc[:, BL:2 * BL], ones_row_f[:], e_f32[:], start=True, stop=True)

                # -- PE: broadcast 1/sum over partitions
                MM(ps_bc[:], ones_row_f[:], rsum[:], start=True, stop=True)

                # -- VEC: xw = exp_s * x_t * bc   (softmax(scores) * x_t)
                y_sb = work.tile([128, BL], MM_DT, tag="y_sb")
                nc.vector.tensor_tensor(y_sb[:], exp_sb[:], xt, op=ALU.mult)
                xw = work.tile([128, BL], MM_DT, tag="xw")
                nc.vector.tensor_tensor(xw[:], y_sb[:], ps_bc[:], op=ALU.mult)

                # -- PE: gi joins the gate accumulations
                for m in range(4):
                    MM(ps_rz[:, m * BL:(m + 1) * BL],
                       wih_m[:, m * 128:(m + 1) * 128], xw[:],
                       start=False, stop=True)
                for m in range(2):
                    MM(ps_in[:, m * BL:(m + 1) * BL],
                       wih_m[:, (4 + m) * 128:(5 + m) * 128], xw[:],
                       start=False, stop=True)

                # -- VEC: U/Z accumulation for current h (enc state s_{t-1})
                if e_f32 is not None:
                    tmp = work.tile([128, 2 * BL], F32, tag="tmp")
                    nc.vector.tensor_tensor(tmp[:], ps_ebc[:], h[:], op=ALU.mult)
                    nc.vector.tensor_tensor(U[:], U[:], tmp[:], op=ALU.add)
                    nc.vector.tensor_tensor(Z[:], Z[:], e_f32[:], op=ALU.add)

                # -- gates: tanh(0.5*(g+b)); sigma(x)=0.5+0.5*tanh(x/2) folded in.
                # r-half first (gates the n-path), z-half runs under the n-path.
                srz = work.tile([128, 4 * BL], F32, tag="srz")
                nc.scalar.activation(srz[:, 0:2 * BL], ps_rz[:, 0:2 * BL],
                                     AF.Tanh, scale=0.5)
                nc.scalar.activation(srz[:, 2 * BL:4 * BL], ps_rz[:, 2 * BL:4 * BL],
                                     AF.Tanh, scale=0.5)
                tr = srz[:, 0:2 * BL]
                tz = srz[:, 2 * BL:4 * BL]

                # -- n = tanh(i_n + r*h_n);  r*h_n = (1+tr) * ps_hn  (whh_n pre-halved)
                nm = work.tile([128, 2 * BL], F32, tag="nm")
                nc.vector.scalar_tensor_tensor(nm[:], tr, 1.0, ps_hn[:],
                                               op0=ALU.add, op1=ALU.mult)
                npre = work.tile([128, 2 * BL], F32, tag="npre")
                nc.vector.tensor_tensor(npre[:], nm[:], ps_in[:], op=ALU.add)
                # z-path precomputes (slack, overlap tanh_n): q=1-z, p2=(1+tz)*h
                q_sb = work.tile([128, 2 * BL], F32, tag="q_sb")
                nc.vector.tensor_scalar(q_sb[:], tz, -0.5, 0.5, ALU.mult, ALU.add)
                p2 = work.tile([128, 2 * BL], F32, tag="p2")
                nc.vector.scalar_tensor_tensor(p2[:], tz, 1.0, h[:],
                                               op0=ALU.add, op1=ALU.mult)
                n_sb = work.tile([128, 2 * BL], F32, tag="n_sb")
                nc.scalar.activation(n_sb[:], npre[:], AF.Tanh)

                # -- h_new = (1-z)*n + z*h = q*n + 0.5*p2
                qn = work.tile([128, 2 * BL], F32, tag="qn")
                nc.vector.tensor_tensor(qn[:], q_sb[:], n_sb[:], op=ALU.mult)
                hm_new = hpool.tile([128, 2 * BL], MM_DT, tag="hm")
                nc.vector.scalar_tensor_tensor(hm_new[:], p2[:], 0.5, qn[:],
                                               op0=ALU.mult, op1=ALU.add)
                h_new = hpool.tile([128, 2 * BL], F32, tag="h")
                nc.vector.scalar_tensor_tensor(h_new[:], p2[:], 0.5, qn[:],
                                               op0=ALU.mult, op1=ALU.add)

                h, hm = h_new, hm_new

            # ---- tail: U-path for the last h ----
            MM(ps_ts[:], wt_m[:, 0:1], hm[:, 0:BL], start=True, stop=False)
            MM(ps_ts[:], wt_m[:, 1:2], hm[:, BL:2 * BL], start=False, stop=True)
            ts_sb = work.tile([1, BL], F32, tag="ts_sb")
            nc.scalar.activation(ts_sb[:], ps_ts[:], AF.Tanh, bias=bt[:, 0:1])
            e_f32 = work.tile([1, BL], F32, tag="e_f32")
            nc.scalar.activation(e_f32[:], ts_sb[:], AF.Exp)
            MM(ps_ebc[:, 0:BL], ones_row_f[:], e_f32[:], start=True, stop=True)
            MM(ps_ebc[:, BL:2 * BL], ones_row_f[:], e_f32[:], start=True, stop=True)
            tmp = work.tile([128, 2 * BL], F32, tag="tmp")
            nc.vector.tensor_tensor(tmp[:], ps_ebc[:], h[:], op=ALU.mult)
            nc.vector.tensor_tensor(U[:], U[:], tmp[:], op=ALU.add)
            nc.vector.tensor_tensor(Z[:], Z[:], e_f32[:], op=ALU.add)

            # ---- context = U / Z ; logits = sigmoid(W_f @ context + b_f) ----
            rZ = work.tile([1, BL], F32, tag="rZ")
            nc.vector.reciprocal(rZ[:], Z[:])
            MM(ps_ebc[:, 0:BL], ones_row_f[:], rZ[:], start=True, stop=True)
            MM(ps_ebc[:, BL:2 * BL], ones_row_f[:], rZ[:], start=True, stop=True)
            ctx = work.tile([128, 2 * BL], MM_DT, tag="ctx")
            nc.vector.tensor_tensor(ctx[:], U[:], ps_ebc[:], op=ALU.mult)
            wf_m = wpool.tile([128, 2], MM_DT, tag="wf_m")
            nc.vector.tensor_copy(wf_m[:], wf[:])
            MM(ps_ts[:], wf_m[:, 0:1], ctx[:, 0:BL], start=True, stop=False)
            MM(ps_ts[:], wf_m[:, 1:2], ctx[:, BL:2 * BL], start=False, stop=True)
            sig_t = work.tile([1, BL], F32, tag="sig_t")
            nc.scalar.activation(sig_t[:], ps_ts[:], AF.Tanh, bias=bf[:, 0:1], scale=0.5)
            out_sb = work.tile([1, BL], F32, tag="out_sb")
            nc.vector.tensor_scalar(out_sb[:], sig_t[:], 0.5, 0.5, ALU.mult, ALU.add)
            nc.sync.dma_start(out_d[:], out_sb[:])

    return nc


_PROGRAM_CACHE = {}


def _get_program(n_steps: int):
    if n_steps not in _PROGRAM_CACHE:
        nc = _build_program(n_steps)
        nc.finalize()
        _PROGRAM_CACHE[n_steps] = nc
    return _PROGRAM_CACHE[n_steps]


def _prep_weights(W_a, b_a, W_ih, b_ih, W_hh, b_hh, W_t, b_t, W_f, b_f):
    f = np.float32
    wa = np.concatenate([W_a[:, 0:128].T, W_a[:, 128:256].T, W_a[:, 256:384].T],
                        axis=1).astype(f)                       # [128, 384]
    wih = np.ascontiguousarray(W_ih.T).astype(f)                # [128, 768]
    W_hh_s = W_hh.copy()
    W_hh_s[512:768, :] = 0.5 * W_hh_s[512:768, :]   # pre-halve n-gate h-side
    whh = np.concatenate([W_hh_s[:, 0:128].T, W_hh_s[:, 128:256].T], axis=1).astype(f)
    brz4 = np.asarray((b_ih + b_hh)[0:512]).reshape(4, 128).astype(f)
    bin2 = np.asarray(b_ih[512:768]).reshape(2, 128).astype(f)
    bhn2 = (0.5 * np.asarray(b_hh[512:768])).reshape(2, 128).astype(f)
    sel4 = np.zeros((4, 128), f)
    for m in range(4):
        sel4[m, m * 32:(m + 1) * 32] = 1.0
    sel2 = np.zeros((2, 64), f)
    for m in range(2):
        sel2[m, m * 32:(m + 1) * 32] = 1.0
    ba = np.asarray(b_a).reshape(128, 1).astype(f)
    wt = W_t.reshape(256).reshape(2, 128).T.astype(f)           # [128, 2]
    bt = np.array([[float(np.asarray(b_t).reshape(()))]], dtype=f)
    wf = W_f.reshape(256).reshape(2, 128).T.astype(f)
    bf = np.array([[0.5 * float(np.asarray(b_f).reshape(()))]], dtype=f)
    return dict(wa=wa, wih=wih, whh=whh, brz4=brz4, bin2=bin2, bhn2=bhn2,
                sel4=sel4, sel2=sel2, ba=ba, wt=wt, bt=bt, wf=wf, bf=bf)


def kernel(x, W_a, b_a, W_ih, b_ih, W_hh, b_hh, W_t, b_t, W_f, b_f,
           n_steps=None, trace=False):
    x = np.asarray(x, dtype=np.float32)
    n_steps = x.shape[1] if n_steps is None else n_steps
    nc = _get_program(n_steps)
    wmap = _prep_weights(np.asarray(W_a), np.asarray(b_a), np.asarray(W_ih),
                         np.asarray(b_ih), np.asarray(W_hh), np.asarray(b_hh),
                         np.asarray(W_t), np.asarray(b_t), np.asarray(W_f),
                         np.asarray(b_f))
    in_maps = []
    nb = x.shape[0] // NCORES
    for c in range(NCORES):
        xc = x[c * nb:(c + 1) * nb]                              # [nb, S, I]
        xf = np.ascontiguousarray(xc.transpose(2, 1, 0)).reshape(128, -1)
        m = dict(wmap)
        m["x"] = np.ascontiguousarray(xf, dtype=np.float32)
        in_maps.append(m)
    res = run_bass_kernel_spmd(nc, in_maps, core_ids=list(range(NCORES)),
                               trace=trace)
    out = np.zeros((x.shape[0], 1), dtype=np.float32)
    for c in range(NCORES):
        out[c * nb:(c + 1) * nb, 0] = res.results[c]["out"][0, :]
    kernel.last_results = res
    return out


# revision 24
# speedup vs baseline: 1.0177x; 1.0177x over previous
"""DA-RNN Trainium2 Bass kernel.

Data-parallel over batch: 256 batch / 8 cores = 32 per core.
Recurrence over S=512 steps runs locally per core, feature-major layout
([feature_partition, batch] tiles). Temporal attention is computed as
running sums during the scan (U = sum_t e_t * h_t, Z = sum_t e_t), so no
encoder buffer is materialized.

All gate/score biases are injected into PSUM by tiny K=1 matmuls on the
(mostly idle) PE, so each activation stage is a single wide ACT op.
sigmoid(x) is computed as 0.5 + 0.5*tanh(x/2) to stay inside the single
exp_and_others ACT table set (exp+tanh) — a set switch costs ~2.7us.
"""

import numpy as np

import concourse.bass as bass
import concourse.mybir as mybir
import concourse.tile as tile
from concourse import bacc
from concourse.bass_utils import run_bass_kernel_spmd

F32 = mybir.dt.float32
BF16 = mybir.dt.bfloat16
AF = mybir.ActivationFunctionType
ALU = mybir.AluOpType

B, S, I, H, O = 256, 512, 128, 256, 1
NCORES = 8
BL = B // NCORES  # 32 local batch

MM_DT = BF16  # dtype for heavy matmul operands (halves LDWEIGHTS via FWL)


def _build_program(n_steps: int):
    nc = bacc.Bacc(None, target_bir_lowering=False)

    # ---- DRAM I/O (per-core shapes; weights replicated across cores) ----
    x_d = nc.dram_tensor("x", [128, n_steps * BL], F32, kind="ExternalInput")
    wa_d = nc.dram_tensor("wa", [128, 3 * 128], F32, kind="ExternalInput")
    wih_d = nc.dram_tensor("wih", [128, 768], F32, kind="ExternalInput")
    whh_d = nc.dram_tensor("whh", [128, 2 * 768], F32, kind="ExternalInput")
    # bias values for PE injection (partition-major rows) + selectors
    brz4_d = nc.dram_tensor("brz4", [4, 128], F32, kind="ExternalInput")
    bin2_d = nc.dram_tensor("bin2", [2, 128], F32, kind="ExternalInput")
    bhn2_d = nc.dram_tensor("bhn2", [2, 128], F32, kind="ExternalInput")
    sel4_d = nc.dram_tensor("sel4", [4, 128], F32, kind="ExternalInput")
    sel2_d = nc.dram_tensor("sel2", [2, 64], F32, kind="ExternalInput")
    ba_d = nc.dram_tensor("ba", [128, 1], F32, kind="ExternalInput")
    wt_d = nc.dram_tensor("wt", [128, 2], F32, kind="ExternalInput")
    bt_d = nc.dram_tensor("bt", [1, 1], F32, kind="ExternalInput")
    wf_d = nc.dram_tensor("wf", [128, 2], F32, kind="ExternalInput")
    bf_d = nc.dram_tensor("bf", [1, 1], F32, kind="ExternalInput")  # 0.5*b_f
    out_d = nc.dram_tensor("out", [1, BL], F32, kind="ExternalOutput")

    with tile.TileContext(nc) as tc:
        with (
            tc.tile_pool(name="big", bufs=1) as big,
            tc.tile_pool(name="wpool", bufs=1) as wpool,
            tc.tile_pool(name="state", bufs=1) as state,
            tc.tile_pool(name="hpool", bufs=2) as hpool,
            tc.tile_pool(name="work", bufs=2) as work,
            tc.tile_pool(name="ps_sc", bufs=1, space="PSUM") as ps_sc_pool,
            tc.tile_pool(name="ps_rz", bufs=1, space="PSUM") as ps_rz_pool,
            tc.tile_pool(name="ps_hn", bufs=1, space="PSUM") as ps_hn_pool,
            tc.tile_pool(name="ps_in", bufs=1, space="PSUM") as ps_in_pool,
            tc.tile_pool(name="ps_sum", bufs=1, space="PSUM") as ps_sum_pool,
            tc.tile_pool(name="ps_bc", bufs=1, space="PSUM") as ps_bc_pool,
            tc.tile_pool(name="ps_ts", bufs=1, space="PSUM") as ps_ts_pool,
            tc.tile_pool(name="ps_ebc", bufs=1, space="PSUM") as ps_ebc_pool,
        ):
            # ACT table warmup: first activation triggers the walrus-inserted
            # table load (needs its own syncs) — keep it dependency-light.
            warm = state.tile([1, 2], F32, tag="warm")
            nc.gpsimd.memset(warm[:], 0.0)
            nc.scalar.activation(warm[:], warm[:], AF.Tanh)
            nc.scalar.activation(warm[:], warm[:], AF.Exp)

            # ---- load inputs into SBUF ----
            x_sb = big.tile([128, n_steps * BL], F32)
            nchunk = 8
            cw = (n_steps * BL) // nchunk
            for c in range(nchunk):
                nc.sync.dma_start(x_sb[:, c * cw:(c + 1) * cw], x_d[:, c * cw:(c + 1) * cw])

            def load_w(dram, shape, name):
                t = wpool.tile(shape, F32, tag=name)
                nc.sync.dma_start(t[:], dram[:])
                return t

            wa = load_w(wa_d, [128, 3 * 128], "wa")
            wih = load_w(wih_d, [128, 768], "wih")
            whh = load_w(whh_d, [128, 2 * 768], "whh")
            brz4 = load_w(brz4_d, [4, 128], "brz4")
            bin2 = load_w(bin2_d, [2, 128], "bin2")
            bhn2 = load_w(bhn2_d, [2, 128], "bhn2")
            sel4 = load_w(sel4_d, [4, 128], "sel4")
            sel2 = load_w(sel2_d, [2, 64], "sel2")
            ba = load_w(ba_d, [128, 1], "ba")
            wt = load_w(wt_d, [128, 2], "wt")
            bt = load_w(bt_d, [1, 1], "bt")
            wf = load_w(wf_d, [128, 2], "wf")
            bf = load_w(bf_d, [1, 1], "bf")

            x_mm = big.tile([128, n_steps * BL], MM_DT)
            for c in range(nchunk):
                nc.vector.tensor_copy(x_mm[:, c * cw:(c + 1) * cw],
                                      x_sb[:, c * cw:(c + 1) * cw])
            wa_m = wpool.tile([128, 3 * 128], MM_DT, tag="wa_m")
            nc.vector.tensor_copy(wa_m[:], wa[:])
            wih_m = wpool.tile([128, 768], MM_DT, tag="wih_m")
            nc.vector.tensor_copy(wih_m[:], wih[:])
            whh_m = wpool.tile([128, 2 * 768], MM_DT, tag="whh_m")
            nc.vector.tensor_copy(whh_m[:], whh[:])
            wt_m = wpool.tile([128, 2], MM_DT, tag="wt_m")
            nc.vector.tensor_copy(wt_m[:], wt[:])
            def to_mm(t, shape, name):
                tm = wpool.tile(shape, MM_DT, tag=name)
                nc.vector.tensor_copy(tm[:], t[:])
                return tm
            brz4_m = to_mm(brz4, [4, 128], "brz4_m")
            bin2_m = to_mm(bin2, [2, 128], "bin2_m")
            bhn2_m = to_mm(bhn2, [2, 128], "bhn2_m")
            sel4_m = to_mm(sel4, [4, 128], "sel4_m")
            sel2_m = to_mm(sel2, [2, 64], "sel2_m")

            ones_col = state.tile([128, 1], MM_DT, tag="ones_col")
            nc.vector.memset(ones_col[:], 1.0)
            ones_row = state.tile([1, 128], MM_DT, tag="ones_row")
            nc.vector.memset(ones_row[:], 1.0)
            ones_row_f = state.tile([1, 128], F32, tag="ones_row_f")
            nc.vector.memset(ones_row_f[:], 1.0)

            # ---- persistent state ----
            U = state.tile([128, 2 * BL], F32, tag="U")
            nc.vector.memset(U[:], 0.0)
            Z = state.tile([1, BL], F32, tag="Z")
            nc.vector.memset(Z[:], 0.0)
            h0_t = hpool.tile([128, 2 * BL], F32, tag="h")
            nc.vector.memset(h0_t[:], 0.0)
            hm0 = hpool.tile([128, 2 * BL], MM_DT, tag="hm")
            nc.vector.memset(hm0[:], 0.0)

            # ---- persistent PSUM tiles (8 banks total) ----
            ps_sc = ps_sc_pool.tile([128, BL], F32)
            ps_rz = ps_rz_pool.tile([128, 4 * BL], F32)
            ps_hn = ps_hn_pool.tile([128, 2 * BL], F32)
            ps_in = ps_in_pool.tile([128, 2 * BL], F32)
            ps_sum = ps_sum_pool.tile([1, BL], F32)
            ps_bc = ps_bc_pool.tile([128, BL], F32)
            ps_ts = ps_ts_pool.tile([1, BL], F32)
            ps_ebc = ps_ebc_pool.tile([128, 2 * BL], F32)

            h = h0_t   # fp32 h (state after previous step)
            hm = hm0   # matmul-dtype h

            MM = nc.tensor.matmul
            ones32 = ones_row[:, 0:BL]

            for t in range(n_steps):
                xt = x_mm[:, t * BL:(t + 1) * BL]

                # -- PE: scores = W_a @ [x_t; h]  (feature-major out [128, BL])
                MM(ps_sc[:], wa_m[:, 0:128], xt, start=True, stop=False)
                MM(ps_sc[:], wa_m[:, 128:256], hm[:, 0:BL], start=False, stop=False)
                MM(ps_sc[:], wa_m[:, 256:384], hm[:, BL:2 * BL], start=False, stop=True)

                # -- U-path part 1: t_score MMs on h (= enc state s_{t-1})
                if t > 0:
                    MM(ps_ts[:], wt_m[:, 0:1], hm[:, 0:BL], start=True, stop=False)
                    MM(ps_ts[:], wt_m[:, 1:2], hm[:, BL:2 * BL], start=False, stop=True)

                # -- PE: bias injections (no data deps — fill PE idle time)
                MM(ps_rz[:], brz4_m[:], sel4_m[:], start=True, stop=False)
                MM(ps_in[:], bin2_m[:], sel2_m[:], start=True, stop=False)
                MM(ps_hn[:], bhn2_m[:], sel2_m[:], start=True, stop=False)

                # -- PE: gh for r,z (accumulate; gi joins later) and for n
                for m in range(4):
                    for k in range(2):
                        MM(ps_rz[:, m * BL:(m + 1) * BL],
                           whh_m[:, k * 768 + m * 128:k * 768 + (m + 1) * 128],
                           hm[:, k * BL:(k + 1) * BL],
                           start=False, stop=False)
                for m in range(2):
                    for k in range(2):
                        MM(ps_hn[:, m * BL:(m + 1) * BL],
                           whh_m[:, k * 768 + (4 + m) * 128:k * 768 + (5 + m) * 128],
                           hm[:, k * BL:(k + 1) * BL],
                           start=False, stop=(k == 1))

                # -- ACT: exp(tanh(scores + b_a)); tanh in-place in PSUM so the
                # exp reads the cheaper PSUM port (172 vs 224 fixed cycles)
                nc.scalar.activation(ps_sc[:], ps_sc[:], AF.Tanh, bias=ba[:, 0:1])
                exp_sb = work.tile([128, BL], MM_DT, tag="exp_sb")
                nc.scalar.activation(exp_sb[:], ps_sc[:], AF.Exp)

                # -- U-path: e = exp(tanh(ts + b_t))
                if t > 0:
                    ts_sb = work.tile([1, BL], F32, tag="ts_sb")
                    nc.scalar.activation(ts_sb[:], ps_ts[:], AF.Tanh, bias=bt[:, 0:1])
                    e_f32 = work.tile([1, BL], F32, tag="e_f32")
                    nc.scalar.activation(e_f32[:], ts_sb[:], AF.Exp)
                else:
                    e_f32 = None

                # -- PE: softmax denominator (partition sum), then VEC recip
                MM(ps_sum[:], ones_col[:], exp_sb[:], start=True, stop=True)
                rsum = work.tile([1, BL], F32, tag="rsum")
                nc.vector.reciprocal(rsum[:], ps_sum[:])

                # -- U-path: broadcast e over partitions
                if e_f32 is not None:
                    MM(ps_ebc[:, 0:BL], ones_row_f[:], e_f32[:], start=True, stop=True)
                    MM(ps_ebc[:, BL:2 * BL], ones_row_f[:], e_f32[:], start=True, stop=True)

                # -- PE: broadcast 1/sum over partitions
                MM(ps_bc[:], ones_row_f[:], rsum[:], start=True, stop=True)

                # -- VEC: xw = exp_s * x_t * bc   (softmax(scores) * x_t)
                y_sb = work.tile([128, BL], MM_DT, tag="y_sb")
                nc.vector.tensor_tensor(y_sb[:], exp_sb[:], xt, op=ALU.mult)
                xw = work.tile([128, BL], MM_DT, tag="xw")
                nc.vector.tensor_tensor(xw[:], y_sb[:], ps_bc[:], op=ALU.mult)

                # -- PE: gi joins the gate accumulations
                for m in range(4):
                    MM(ps_rz[:, m * BL:(m + 1) * BL],
                       wih_m[:, m * 128:(m + 1) * 128], xw[:],
                       start=False, stop=True)
                for m in range(2):
                    MM(ps_in[:, m * BL:(m + 1) * BL],
                       wih_m[:, (4 + m) * 128:(5 + m) * 128], xw[:],
                       start=False, stop=True)

                # -- VEC: U/Z accumulation for current h (enc state s_{t-1})
                if e_f32 is not None:
                    tmp = work.tile([128, 2 * BL], F32, tag="tmp")
                    nc.vector.tensor_tensor(tmp[:], ps_ebc[:], h[:], op=ALU.mult)
                    nc.vector.tensor_tensor(U[:], U[:], tmp[:], op=ALU.add)
                    nc.vector.tensor_tensor(Z[:], Z[:], e_f32[:], op=ALU.add)

                # -- gates: tanh(0.5*(g+b)); sigma(x)=0.5+0.5*tanh(x/2) folded in.
                # r-half first (gates the n-path), z-half runs under the n-path.
                srz = work.tile([128, 4 * BL], F32, tag="srz")
                nc.scalar.activation(srz[:, 0:2 * BL], ps_rz[:, 0:2 * BL],
                                     AF.Tanh, scale=0.5)
                nc.scalar.activation(srz[:, 2 * BL:4 * BL], ps_rz[:, 2 * BL:4 * BL],
                                     AF.Tanh, scale=0.5)
                tr = srz[:, 0:2 * BL]
                tz = srz[:, 2 * BL:4 * BL]

                # -- n = tanh(i_n + r*h_n);  r*h_n = (1+tr) * ps_hn  (whh_n pre-halved)
                nm = work.tile([128, 2 * BL], F32, tag="nm")
                nc.vector.scalar_tensor_tensor(nm[:], tr, 1.0, ps_hn[:],
                                               op0=ALU.add, op1=ALU.mult)
                npre = ps_ebc  # reuse: ebc consumed by the U-path already
                nc.vector.tensor_tensor(npre[:], nm[:], ps_in[:], op=ALU.add)
                # z-path precomputes (slack, overlap tanh_n): q=1-z, p2=(1+tz)*h
                q_sb = work.tile([128, 2 * BL], BF16, tag="q_sb")
                nc.vector.tensor_scalar(q_sb[:], tz, -0.5, 0.5, ALU.mult, ALU.add)
                p2 = work.tile([128, 2 * BL], BF16, tag="p2")
                nc.vector.scalar_tensor_tensor(p2[:], tz, 1.0, h[:],
                                               op0=ALU.add, op1=ALU.mult)
                n_sb = work.tile([128, 2 * BL], BF16, tag="n_sb")
                nc.scalar.activation(n_sb[:], npre[:], AF.Tanh)

                # -- h_new = (1-z)*n + z*h = q*n + 0.5*p2
                qn = work.tile([128, 2 * BL], BF16, tag="qn")
                nc.vector.tensor_tensor(qn[:], q_sb[:], n_sb[:], op=ALU.mult)
                hm_new = hpool.tile([128, 2 * BL], MM_DT, tag="hm")
                nc.vector.scalar_tensor_tensor(hm_new[:], p2[:], 0.5, qn[:],
                                               op0=ALU.mult, op1=ALU.add)
                h_new = hpool.tile([128, 2 * BL], F32, tag="h")
                nc.vector.scalar_tensor_tensor(h_new[:], p2[:], 0.5, qn[:],
                                               op0=ALU.mult, op1=ALU.add)

                h, hm = h_new, hm_new

            # ---- tail: U-path for the last h ----
            MM(ps_ts[:], wt_m[:, 0:1], hm[:, 0:BL], start=True, stop=False)
            MM(ps_ts[:], wt_m[:, 1:2], hm[:, BL:2 * BL], start=False, stop=True)
            ts_sb = work.tile([1, BL], F32, tag="ts_sb")
            nc.scalar.activation(ts_sb[:], ps_ts[:], AF.Tanh, bias=bt[:, 0:1])
            e_f32 = work.tile([1, BL], F32, tag="e_f32")
            nc.scalar.activation(e_f32[:], ts_sb[:], AF.Exp)
            MM(ps_ebc[:, 0:BL], ones_row_f[:], e_f32[:], start=True, stop=True)
            MM(ps_ebc[:, BL:2 * BL], ones_row_f[:], e_f32[:], start=True, stop=True)
            tmp = work.tile([128, 2 * BL], F32, tag="tmp")
            nc.vector.tensor_tensor(tmp[:], ps_ebc[:], h[:], op=ALU.mult)
            nc.vector.tensor_tensor(U[:], U[:], tmp[:], op=ALU.add)
            nc.vector.tensor_tensor(Z[:], Z[:], e_f32[:], op=ALU.add)

            # ---- context = U / Z ; logits = sigmoid(W_f @ context + b_f) ----
            rZ = work.tile([1, BL], F32, tag="rZ")
            nc.vector.reciprocal(rZ[:], Z[:])
            MM(ps_ebc[:, 0:BL], ones_row_f[:], rZ[:], start=True, stop=True)
            MM(ps_ebc[:, BL:2 * BL], ones_row_f[:], rZ[:], start=True, stop=True)
            ctx = work.tile([128, 2 * BL], MM_DT, tag="ctx")
            nc.vector.tensor_tensor(ctx[:], U[:], ps_ebc[:], op=ALU.mult)
            wf_m = wpool.tile([128, 2], MM_DT, tag="wf_m")
            nc.vector.tensor_copy(wf_m[:], wf[:])
            MM(ps_ts[:], wf_m[:, 0:1], ctx[:, 0:BL], start=True, stop=False)
            MM(ps_ts[:], wf_m[:, 1:2], ctx[:, BL:2 * BL], start=False, stop=True)
            sig_t = work.tile([1, BL], F32, tag="sig_t")
            nc.scalar.activation(sig_t[:], ps_ts[:], AF.Tanh, bias=bf[:, 0:1], scale=0.5)
            out_sb = work.tile([1, BL], F32, tag="out_sb")
            nc.vector.tensor_scalar(out_sb[:], sig_t[:], 0.5, 0.5, ALU.mult, ALU.add)
            nc.sync.dma_start(out_d[:], out_sb[:])

    return nc


_PROGRAM_CACHE = {}


def _get_program(n_steps: int):
    if n_steps not in _PROGRAM_CACHE:
        nc = _build_program(n_steps)
        nc.finalize()
        _PROGRAM_CACHE[n_steps] = nc
    return _PROGRAM_CACHE[n_steps]


def _prep_weights(W_a, b_a, W_ih, b_ih, W_hh, b_hh, W_t, b_t, W_f, b_f):
    f = np.float32
    wa = np.concatenate([W_a[:, 0:128].T, W_a[:, 128:256].T, W_a[:, 256:384].T],
                        axis=1).astype(f)                       # [128, 384]
    wih = np.ascontiguousarray(W_ih.T).astype(f)                # [128, 768]
    W_hh_s = W_hh.copy()
    W_hh_s[512:768, :] = 0.5 * W_hh_s[512:768, :]   # pre-halve n-gate h-side
    whh = np.concatenate([W_hh_s[:, 0:128].T, W_hh_s[:, 128:256].T], axis=1).astype(f)
    brz4 = np.asarray((b_ih + b_hh)[0:512]).reshape(4, 128).astype(f)
    bin2 = np.asarray(b_ih[512:768]).reshape(2, 128).astype(f)
    bhn2 = (0.5 * np.asarray(b_hh[512:768])).reshape(2, 128).astype(f)
    sel4 = np.zeros((4, 128), f)
    for m in range(4):
        sel4[m, m * 32:(m + 1) * 32] = 1.0
    sel2 = np.zeros((2, 64), f)
    for m in range(2):
        sel2[m, m * 32:(m + 1) * 32] = 1.0
    ba = np.asarray(b_a).reshape(128, 1).astype(f)
    wt = W_t.reshape(256).reshape(2, 128).T.astype(f)           # [128, 2]
    bt = np.array([[float(np.asarray(b_t).reshape(()))]], dtype=f)
    wf = W_f.reshape(256).reshape(2, 128).T.astype(f)
    bf = np.array([[0.5 * float(np.asarray(b_f).reshape(()))]], dtype=f)
    return dict(wa=wa, wih=wih, whh=whh, brz4=brz4, bin2=bin2, bhn2=bhn2,
                sel4=sel4, sel2=sel2, ba=ba, wt=wt, bt=bt, wf=wf, bf=bf)


def kernel(x, W_a, b_a, W_ih, b_ih, W_hh, b_hh, W_t, b_t, W_f, b_f,
           n_steps=None, trace=False):
    x = np.asarray(x, dtype=np.float32)
    n_steps = x.shape[1] if n_steps is None else n_steps
    nc = _get_program(n_steps)
    wmap = _prep_weights(np.asarray(W_a), np.asarray(b_a), np.asarray(W_ih),
                         np.asarray(b_ih), np.asarray(W_hh), np.asarray(b_hh),
                         np.asarray(W_t), np.asarray(b_t), np.asarray(W_f),
                         np.asarray(b_f))
    in_maps = []
    nb = x.shape[0] // NCORES
    for c in range(NCORES):
        xc = x[c * nb:(c + 1) * nb]                              # [nb, S, I]
        xf = np.ascontiguousarray(xc.transpose(2, 1, 0)).reshape(128, -1)
        m = dict(wmap)
        m["x"] = np.ascontiguousarray(xf, dtype=np.float32)
        in_maps.append(m)
    res = run_bass_kernel_spmd(nc, in_maps, core_ids=list(range(NCORES)),
                               trace=trace)
    out = np.zeros((x.shape[0], 1), dtype=np.float32)
    for c in range(NCORES):
        out[c * nb:(c + 1) * nb, 0] = res.results[c]["out"][0, :]
    kernel.last_results = res
    return out


# revision 25
# speedup vs baseline: 1.0254x; 1.0076x over previous
"""DA-RNN Trainium2 Bass kernel.

Data-parallel over batch: 256 batch / 8 cores = 32 per core.
Recurrence over S=512 steps runs locally per core, feature-major layout
([feature_partition, batch] tiles). Temporal attention is computed as
running sums during the scan (U = sum_t e_t * h_t, Z = sum_t e_t), so no
encoder buffer is materialized.

All gate/score biases are injected into PSUM by tiny K=1 matmuls on the
(mostly idle) PE, so each activation stage is a single wide ACT op.
sigmoid(x) is computed as 0.5 + 0.5*tanh(x/2) to stay inside the single
exp_and_others ACT table set (exp+tanh) — a set switch costs ~2.7us.
"""

import numpy as np

import concourse.bass as bass
import concourse.mybir as mybir
import concourse.tile as tile
from concourse import bacc
from concourse.bass_utils import run_bass_kernel_spmd
from concourse.tile_rust import add_dep_helper

F32 = mybir.dt.float32
BF16 = mybir.dt.bfloat16
AF = mybir.ActivationFunctionType
ALU = mybir.AluOpType

B, S, I, H, O = 256, 512, 128, 256, 1
NCORES = 8
BL = B // NCORES  # 32 local batch

MM_DT = BF16  # dtype for heavy matmul operands (halves LDWEIGHTS via FWL)


def _build_program(n_steps: int):
    nc = bacc.Bacc(None, target_bir_lowering=False)

    # ---- DRAM I/O (per-core shapes; weights replicated across cores) ----
    x_d = nc.dram_tensor("x", [128, n_steps * BL], F32, kind="ExternalInput")
    wa_d = nc.dram_tensor("wa", [128, 3 * 128], F32, kind="ExternalInput")
    wih_d = nc.dram_tensor("wih", [128, 768], F32, kind="ExternalInput")
    whh_d = nc.dram_tensor("whh", [128, 2 * 768], F32, kind="ExternalInput")
    # bias values for PE injection (partition-major rows) + selectors
    brz4_d = nc.dram_tensor("brz4", [4, 128], F32, kind="ExternalInput")
    bin2_d = nc.dram_tensor("bin2", [2, 128], F32, kind="ExternalInput")
    bhn2_d = nc.dram_tensor("bhn2", [2, 128], F32, kind="ExternalInput")
    sel4_d = nc.dram_tensor("sel4", [4, 128], F32, kind="ExternalInput")
    sel2_d = nc.dram_tensor("sel2", [2, 64], F32, kind="ExternalInput")
    ba_d = nc.dram_tensor("ba", [128, 1], F32, kind="ExternalInput")
    wt_d = nc.dram_tensor("wt", [128, 2], F32, kind="ExternalInput")
    bt_d = nc.dram_tensor("bt", [1, 1], F32, kind="ExternalInput")
    wf_d = nc.dram_tensor("wf", [128, 2], F32, kind="ExternalInput")
    bf_d = nc.dram_tensor("bf", [1, 1], F32, kind="ExternalInput")  # 0.5*b_f
    out_d = nc.dram_tensor("out", [1, BL], F32, kind="ExternalOutput")

    with tile.TileContext(nc) as tc:
        with (
            tc.tile_pool(name="big", bufs=1) as big,
            tc.tile_pool(name="wpool", bufs=1) as wpool,
            tc.tile_pool(name="state", bufs=1) as state,
            tc.tile_pool(name="hpool", bufs=2) as hpool,
            tc.tile_pool(name="work", bufs=2) as work,
            tc.tile_pool(name="ps_sc", bufs=1, space="PSUM") as ps_sc_pool,
            tc.tile_pool(name="ps_rz", bufs=1, space="PSUM") as ps_rz_pool,
            tc.tile_pool(name="ps_hn", bufs=1, space="PSUM") as ps_hn_pool,
            tc.tile_pool(name="ps_in", bufs=1, space="PSUM") as ps_in_pool,
            tc.tile_pool(name="ps_sum", bufs=1, space="PSUM") as ps_sum_pool,
            tc.tile_pool(name="ps_bc", bufs=1, space="PSUM") as ps_bc_pool,
            tc.tile_pool(name="ps_ts", bufs=1, space="PSUM") as ps_ts_pool,
            tc.tile_pool(name="ps_ebc", bufs=1, space="PSUM") as ps_ebc_pool,
        ):
            # ACT table warmup: first activation triggers the walrus-inserted
            # table load (needs its own syncs) — keep it dependency-light.
            warm = state.tile([1, 2], F32, tag="warm")
            nc.gpsimd.memset(warm[:], 0.0)
            nc.scalar.activation(warm[:], warm[:], AF.Tanh)
            nc.scalar.activation(warm[:], warm[:], AF.Exp)

            # ---- load inputs into SBUF ----
            x_sb = big.tile([128, n_steps * BL], F32)
            nchunk = 8
            cw = (n_steps * BL) // nchunk
            for c in range(nchunk):
                nc.sync.dma_start(x_sb[:, c * cw:(c + 1) * cw], x_d[:, c * cw:(c + 1) * cw])

            def load_w(dram, shape, name):
                t = wpool.tile(shape, F32, tag=name)
                nc.sync.dma_start(t[:], dram[:])
                return t

            wa = load_w(wa_d, [128, 3 * 128], "wa")
            wih = load_w(wih_d, [128, 768], "wih")
            whh = load_w(whh_d, [128, 2 * 768], "whh")
            brz4 = load_w(brz4_d, [4, 128], "brz4")
            bin2 = load_w(bin2_d, [2, 128], "bin2")
            bhn2 = load_w(bhn2_d, [2, 128], "bhn2")
            sel4 = load_w(sel4_d, [4, 128], "sel4")
            sel2 = load_w(sel2_d, [2, 64], "sel2")
            ba = load_w(ba_d, [128, 1], "ba")
            wt = load_w(wt_d, [128, 2], "wt")
            bt = load_w(bt_d, [1, 1], "bt")
            wf = load_w(wf_d, [128, 2], "wf")
            bf = load_w(bf_d, [1, 1], "bf")

            x_mm = big.tile([128, n_steps * BL], MM_DT)
            for c in range(nchunk):
                nc.vector.tensor_copy(x_mm[:, c * cw:(c + 1) * cw],
                                      x_sb[:, c * cw:(c + 1) * cw])
            wa_m = wpool.tile([128, 3 * 128], MM_DT, tag="wa_m")
            nc.vector.tensor_copy(wa_m[:], wa[:])
            wih_m = wpool.tile([128, 768], MM_DT, tag="wih_m")
            nc.vector.tensor_copy(wih_m[:], wih[:])
            whh_m = wpool.tile([128, 2 * 768], MM_DT, tag="whh_m")
            nc.vector.tensor_copy(whh_m[:], whh[:])
            wt_m = wpool.tile([128, 2], MM_DT, tag="wt_m")
            nc.vector.tensor_copy(wt_m[:], wt[:])
            def to_mm(t, shape, name):
                tm = wpool.tile(shape, MM_DT, tag=name)
                nc.vector.tensor_copy(tm[:], t[:])
                return tm
            brz4_m = to_mm(brz4, [4, 128], "brz4_m")
            bin2_m = to_mm(bin2, [2, 128], "bin2_m")
            bhn2_m = to_mm(bhn2, [2, 128], "bhn2_m")
            sel4_m = to_mm(sel4, [4, 128], "sel4_m")
            sel2_m = to_mm(sel2, [2, 64], "sel2_m")

            ones_col = state.tile([128, 1], MM_DT, tag="ones_col")
            nc.vector.memset(ones_col[:], 1.0)
            ones_row = state.tile([1, 128], MM_DT, tag="ones_row")
            nc.vector.memset(ones_row[:], 1.0)
            ones_row_f = state.tile([1, 128], F32, tag="ones_row_f")
            nc.vector.memset(ones_row_f[:], 1.0)

            # ---- persistent state ----
            U = state.tile([128, 2 * BL], F32, tag="U")
            nc.vector.memset(U[:], 0.0)
            Z = state.tile([1, BL], F32, tag="Z")
            nc.vector.memset(Z[:], 0.0)
            h0_t = hpool.tile([128, 2 * BL], F32, tag="h")
            nc.vector.memset(h0_t[:], 0.0)
            hm0 = hpool.tile([128, 2 * BL], MM_DT, tag="hm")
            nc.vector.memset(hm0[:], 0.0)

            # ---- persistent PSUM tiles (8 banks total) ----
            ps_sc = ps_sc_pool.tile([128, BL], F32)
            ps_rz = ps_rz_pool.tile([128, 4 * BL], F32)
            ps_hn = ps_hn_pool.tile([128, 2 * BL], F32)
            ps_in = ps_in_pool.tile([128, 2 * BL], F32)
            ps_sum = ps_sum_pool.tile([1, BL], F32)
            ps_bc = ps_bc_pool.tile([128, BL], F32)
            ps_ts = ps_ts_pool.tile([1, BL], F32)
            ps_ebc = ps_ebc_pool.tile([128, 2 * BL], F32)

            h = h0_t   # fp32 h (state after previous step)
            hm = hm0   # matmul-dtype h

            MM = nc.tensor.matmul
            ones32 = ones_row[:, 0:BL]

            for t in range(n_steps):
                xt = x_mm[:, t * BL:(t + 1) * BL]

                # -- PE: scores = W_a @ [x_t; h]  (feature-major out [128, BL])
                MM(ps_sc[:], wa_m[:, 0:128], xt, start=True, stop=False)
                MM(ps_sc[:], wa_m[:, 128:256], hm[:, 0:BL], start=False, stop=False)
                MM(ps_sc[:], wa_m[:, 256:384], hm[:, BL:2 * BL], start=False, stop=True)

                # -- U-path part 1: t_score MMs on h (= enc state s_{t-1})
                if t > 0:
                    MM(ps_ts[:], wt_m[:, 0:1], hm[:, 0:BL], start=True, stop=False)
                    MM(ps_ts[:], wt_m[:, 1:2], hm[:, BL:2 * BL], start=False, stop=True)

                # -- PE: bias injections (no data deps — fill PE idle time)
                MM(ps_rz[:], brz4_m[:], sel4_m[:], start=True, stop=False)
                MM(ps_in[:], bin2_m[:], sel2_m[:], start=True, stop=False)
                MM(ps_hn[:], bhn2_m[:], sel2_m[:], start=True, stop=False)

                # -- PE: gh for r,z (accumulate; gi joins later) and for n
                for m in range(4):
                    for k in range(2):
                        MM(ps_rz[:, m * BL:(m + 1) * BL],
                           whh_m[:, k * 768 + m * 128:k * 768 + (m + 1) * 128],
                           hm[:, k * BL:(k + 1) * BL],
                           start=False, stop=False)
                for m in range(2):
                    for k in range(2):
                        MM(ps_hn[:, m * BL:(m + 1) * BL],
                           whh_m[:, k * 768 + (4 + m) * 128:k * 768 + (5 + m) * 128],
                           hm[:, k * BL:(k + 1) * BL],
                           start=False, stop=(k == 1))

                # -- ACT: exp(tanh(scores + b_a)); tanh in-place in PSUM so the
                # exp reads the cheaper PSUM port (172 vs 224 fixed cycles)
                nc.scalar.activation(ps_sc[:], ps_sc[:], AF.Tanh, bias=ba[:, 0:1])
                exp_sb = work.tile([128, BL], MM_DT, tag="exp_sb")
                exp_inst = nc.scalar.activation(exp_sb[:], ps_sc[:], AF.Exp)

                # -- U-path: e = exp(tanh(ts + b_t)); keep it BEHIND exp_s on
                # the ACT queue (the heap otherwise reorders it onto the chain)
                if t > 0:
                    ts_sb = work.tile([1, BL], F32, tag="ts_sb")
                    ts_inst = nc.scalar.activation(ts_sb[:], ps_ts[:], AF.Tanh,
                                                   bias=bt[:, 0:1])
                    add_dep_helper(ts_inst.ins, exp_inst.ins, sync=False,
                                   reason="ts pair after chain exp")
                    e_f32 = work.tile([1, BL], F32, tag="e_f32")
                    nc.scalar.activation(e_f32[:], ts_sb[:], AF.Exp)
                else:
                    e_f32 = None

                # -- PE: softmax denominator (partition sum), then VEC recip
                MM(ps_sum[:], ones_col[:], exp_sb[:], start=True, stop=True)
                rsum = work.tile([1, BL], F32, tag="rsum")
                nc.vector.reciprocal(rsum[:], ps_sum[:])

                # -- U-path: broadcast e over partitions
                if e_f32 is not None:
                    MM(ps_ebc[:, 0:BL], ones_row_f[:], e_f32[:], start=True, stop=True)
                    MM(ps_ebc[:, BL:2 * BL], ones_row_f[:], e_f32[:], start=True, stop=True)

                # -- PE: broadcast 1/sum over partitions
                MM(ps_bc[:], ones_row_f[:], rsum[:], start=True, stop=True)

                # -- VEC: xw = exp_s * x_t * bc   (softmax(scores) * x_t)
                y_sb = work.tile([128, BL], MM_DT, tag="y_sb")
                nc.vector.tensor_tensor(y_sb[:], exp_sb[:], xt, op=ALU.mult)
                xw = work.tile([128, BL], MM_DT, tag="xw")
                nc.vector.tensor_tensor(xw[:], y_sb[:], ps_bc[:], op=ALU.mult)

                # -- PE: gi joins the gate accumulations
                for m in range(4):
                    MM(ps_rz[:, m * BL:(m + 1) * BL],
                       wih_m[:, m * 128:(m + 1) * 128], xw[:],
                       start=False, stop=True)
                for m in range(2):
                    MM(ps_in[:, m * BL:(m + 1) * BL],
                       wih_m[:, (4 + m) * 128:(5 + m) * 128], xw[:],
                       start=False, stop=True)

                # -- VEC: U/Z accumulation for current h (enc state s_{t-1})
                if e_f32 is not None:
                    tmp = work.tile([128, 2 * BL], F32, tag="tmp")
                    nc.vector.tensor_tensor(tmp[:], ps_ebc[:], h[:], op=ALU.mult)
                    nc.vector.tensor_tensor(U[:], U[:], tmp[:], op=ALU.add)
                    nc.vector.tensor_tensor(Z[:], Z[:], e_f32[:], op=ALU.add)

                # -- gates: tanh(0.5*(g+b)); sigma(x)=0.5+0.5*tanh(x/2) folded in.
                # r-half first (gates the n-path), z-half runs under the n-path.
                srz = work.tile([128, 4 * BL], F32, tag="srz")
                nc.scalar.activation(srz[:, 0:2 * BL], ps_rz[:, 0:2 * BL],
                                     AF.Tanh, scale=0.5)
                nc.scalar.activation(srz[:, 2 * BL:4 * BL], ps_rz[:, 2 * BL:4 * BL],
                                     AF.Tanh, scale=0.5)
                tr = srz[:, 0:2 * BL]
                tz = srz[:, 2 * BL:4 * BL]

                # -- n = tanh(i_n + r*h_n);  r*h_n = (1+tr) * ps_hn  (whh_n pre-halved)
                nm = work.tile([128, 2 * BL], F32, tag="nm")
                nc.vector.scalar_tensor_tensor(nm[:], tr, 1.0, ps_hn[:],
                                               op0=ALU.add, op1=ALU.mult)
                npre = ps_ebc  # reuse: ebc consumed by the U-path already
                nc.vector.tensor_tensor(npre[:], nm[:], ps_in[:], op=ALU.add)
                # z-path precomputes (slack, overlap tanh_n): q=1-z, p2=(1+tz)*h
                q_sb = work.tile([128, 2 * BL], BF16, tag="q_sb")
                nc.vector.tensor_scalar(q_sb[:], tz, -0.5, 0.5, ALU.mult, ALU.add)
                p2 = work.tile([128, 2 * BL], BF16, tag="p2")
                nc.vector.scalar_tensor_tensor(p2[:], tz, 1.0, h[:],
                                               op0=ALU.add, op1=ALU.mult)
                n_sb = work.tile([128, 2 * BL], BF16, tag="n_sb")
                nc.scalar.activation(n_sb[:], npre[:], AF.Tanh)

                # -- h_new = (1-z)*n + z*h = q*n + 0.5*p2
                qn = work.tile([128, 2 * BL], BF16, tag="qn")
                nc.vector.tensor_tensor(qn[:], q_sb[:], n_sb[:], op=ALU.mult)
                hm_new = hpool.tile([128, 2 * BL], MM_DT, tag="hm")
                nc.vector.scalar_tensor_tensor(hm_new[:], p2[:], 0.5, qn[:],
                                               op0=ALU.mult, op1=ALU.add)
                h_new = hpool.tile([128, 2 * BL], F32, tag="h")
                nc.vector.scalar_tensor_tensor(h_new[:], p2[:], 0.5, qn[:],
                                               op0=ALU.mult, op1=ALU.add)

                h, hm = h_new, hm_new

            # ---- tail: U-path for the last h ----
            MM(ps_ts[:], wt_m[:, 0:1], hm[:, 0:BL], start=True, stop=False)
            MM(ps_ts[:], wt_m[:, 1:2], hm[:, BL:2 * BL], start=False, stop=True)
            ts_sb = work.tile([1, BL], F32, tag="ts_sb")
            nc.scalar.activation(ts_sb[:], ps_ts[:], AF.Tanh, bias=bt[:, 0:1])
            e_f32 = work.tile([1, BL], F32, tag="e_f32")
            nc.scalar.activation(e_f32[:], ts_sb[:], AF.Exp)
            MM(ps_ebc[:, 0:BL], ones_row_f[:], e_f32[:], start=True, stop=True)
            MM(ps_ebc[:, BL:2 * BL], ones_row_f[:], e_f32[:], start=True, stop=True)
            tmp = work.tile([128, 2 * BL], F32, tag="tmp")
            nc.vector.tensor_tensor(tmp[:], ps_ebc[:], h[:], op=ALU.mult)
            nc.vector.tensor_tensor(U[:], U[:], tmp[:], op=ALU.add)
            nc.vector.tensor_tensor(Z[:], Z[:], e_f32[:], op=ALU.add)

            # ---- context = U / Z ; logits = sigmoid(W_f @ context + b_f) ----
            rZ = work.tile([1, BL], F32, tag="rZ")
            nc.vector.reciprocal(rZ[:], Z[:])
            MM(ps_ebc[:, 0:BL], ones_row_f[:], rZ[:], start=True, stop=True)
            MM(ps_ebc[:, BL:2 * BL], ones_row_f[:], rZ[:], start=True, stop=True)
            ctx = work.tile([128, 2 * BL], MM_DT, tag="ctx")
            nc.vector.tensor_tensor(ctx[:], U[:], ps_ebc[:], op=ALU.mult)
            wf_m = wpool.tile([128, 2], MM_DT, tag="wf_m")
            nc.vector.tensor_copy(wf_m[:], wf[:])
            MM(ps_ts[:], wf_m[:, 0:1], ctx[:, 0:BL], start=True, stop=False)
            MM(ps_ts[:], wf_m[:, 1:2], ctx[:, BL:2 * BL], start=False, stop=True)
            sig_t = work.tile([1, BL], F32, tag="sig_t")
            nc.scalar.activation(sig_t[:], ps_ts[:], AF.Tanh, bias=bf[:, 0:1], scale=0.5)
            out_sb = work.tile([1, BL], F32, tag="out_sb")
            nc.vector.tensor_scalar(out_sb[:], sig_t[:], 0.5, 0.5, ALU.mult, ALU.add)
            nc.sync.dma_start(out_d[:], out_sb[:])

    return nc


_PROGRAM_CACHE = {}


def _get_program(n_steps: int):
    if n_steps not in _PROGRAM_CACHE:
        nc = _build_program(n_steps)
        nc.finalize()
        _PROGRAM_CACHE[n_steps] = nc
    return _PROGRAM_CACHE[n_steps]


def _prep_weights(W_a, b_a, W_ih, b_ih, W_hh, b_hh, W_t, b_t, W_f, b_f):
    f = np.float32
    wa = np.concatenate([W_a[:, 0:128].T, W_a[:, 128:256].T, W_a[:, 256:384].T],
                        axis=1).astype(f)                       # [128, 384]
    wih = np.ascontiguousarray(W_ih.T).astype(f)                # [128, 768]
    W_hh_s = W_hh.copy()
    W_hh_s[512:768, :] = 0.5 * W_hh_s[512:768, :]   # pre-halve n-gate h-side
    whh = np.concatenate([W_hh_s[:, 0:128].T, W_hh_s[:, 128:256].T], axis=1).astype(f)
    brz4 = np.asarray((b_ih + b_hh)[0:512]).reshape(4, 128).astype(f)
    bin2 = np.asarray(b_ih[512:768]).reshape(2, 128).astype(f)
    bhn2 = (0.5 * np.asarray(b_hh[512:768])).reshape(2, 128).astype(f)
    sel4 = np.zeros((4, 128), f)
    for m in range(4):
        sel4[m, m * 32:(m + 1) * 32] = 1.0
    sel2 = np.zeros((2, 64), f)
    for m in range(2):
        sel2[m, m * 32:(m + 1) * 32] = 1.0
    ba = np.asarray(b_a).reshape(128, 1).astype(f)
    wt = W_t.reshape(256).reshape(2, 128).T.astype(f)           # [128, 2]
    bt = np.array([[float(np.asarray(b_t).reshape(()))]], dtype=f)
    wf = W_f.reshape(256).reshape(2, 128).T.astype(f)
    bf = np.array([[0.5 * float(np.asarray(b_f).reshape(()))]], dtype=f)
    return dict(wa=wa, wih=wih, whh=whh, brz4=brz4, bin2=bin2, bhn2=bhn2,
                sel4=sel4, sel2=sel2, ba=ba, wt=wt, bt=bt, wf=wf, bf=bf)


def kernel(x, W_a, b_a, W_ih, b_ih, W_hh, b_hh, W_t, b_t, W_f, b_f,
           n_steps=None, trace=False):
    x = np.asarray(x, dtype=np.float32)
    n_steps = x.shape[1] if n_steps is None else n_steps
    nc = _get_program(n_steps)
    wmap = _prep_weights(np.asarray(W_a), np.asarray(b_a), np.asarray(W_ih),
                         np.asarray(b_ih), np.asarray(W_hh), np.asarray(b_hh),
                         np.asarray(W_t), np.asarray(b_t), np.asarray(W_f),
                         np.asarray(b_f))
    in_maps = []
    nb = x.shape[0] // NCORES
    for c in range(NCORES):
        xc = x[c * nb:(c + 1) * nb]                              # [nb, S, I]
        xf = np.ascontiguousarray(xc.transpose(2, 1, 0)).reshape(128, -1)
        m = dict(wmap)
        m["x"] = np.ascontiguousarray(xf, dtype=np.float32)
        in_maps.append(m)
    res = run_bass_kernel_spmd(nc, in_maps, core_ids=list(range(NCORES)),
                               trace=trace)
    out = np.zeros((x.shape[0], 1), dtype=np.float32)
    for c in range(NCORES):
        out[c * nb:(c + 1) * nb, 0] = res.results[c]["out"][0, :]
    kernel.last_results = res
    return out


# revision 28
# speedup vs baseline: 1.0552x; 1.0290x over previous
"""DA-RNN Trainium2 Bass kernel.

Data-parallel over batch: 256 batch / 8 cores = 32 per core.
Recurrence over S=512 steps runs locally per core, feature-major layout
([feature_partition, batch] tiles). Temporal attention is computed as
running sums during the scan (U = sum_t e_t * h_t, Z = sum_t e_t), so no
encoder buffer is materialized.

All gate/score biases are injected into PSUM by tiny K=1 matmuls on the
(mostly idle) PE, so each activation stage is a single wide ACT op.
sigmoid(x) is computed as 0.5 + 0.5*tanh(x/2) to stay inside the single
exp_and_others ACT table set (exp+tanh) — a set switch costs ~2.7us.
"""

import numpy as np

import concourse.bass as bass
import concourse.mybir as mybir
import concourse.tile as tile
from concourse import bacc
from concourse.bass_utils import run_bass_kernel_spmd
from concourse.tile_rust import add_dep_helper

F32 = mybir.dt.float32
BF16 = mybir.dt.bfloat16
AF = mybir.ActivationFunctionType
ALU = mybir.AluOpType

B, S, I, H, O = 256, 512, 128, 256, 1
NCORES = 8
BL = B // NCORES  # 32 local batch

MM_DT = BF16  # dtype for heavy matmul operands (halves LDWEIGHTS via FWL)


def _build_program(n_steps: int):
    nc = bacc.Bacc(None, target_bir_lowering=False)

    # ---- DRAM I/O (per-core shapes; weights replicated across cores) ----
    x_d = nc.dram_tensor("x", [128, n_steps * BL], F32, kind="ExternalInput")
    wa_d = nc.dram_tensor("wa", [128, 3 * 128], F32, kind="ExternalInput")
    wih_d = nc.dram_tensor("wih", [128, 768], F32, kind="ExternalInput")
    whh_d = nc.dram_tensor("whh", [128, 2 * 768], F32, kind="ExternalInput")
    # bias values for PE injection (partition-major rows) + selectors
    brz4_d = nc.dram_tensor("brz4", [4, 128], F32, kind="ExternalInput")
    bin2_d = nc.dram_tensor("bin2", [2, 128], F32, kind="ExternalInput")
    bhn2_d = nc.dram_tensor("bhn2", [2, 128], F32, kind="ExternalInput")
    sel4_d = nc.dram_tensor("sel4", [4, 128], F32, kind="ExternalInput")
    sel2_d = nc.dram_tensor("sel2", [2, 64], F32, kind="ExternalInput")
    ba_d = nc.dram_tensor("ba", [128, 1], F32, kind="ExternalInput")
    wt_d = nc.dram_tensor("wt", [128, 2], F32, kind="ExternalInput")
    bt_d = nc.dram_tensor("bt", [1, 1], F32, kind="ExternalInput")
    wf_d = nc.dram_tensor("wf", [128, 2], F32, kind="ExternalInput")
    bf_d = nc.dram_tensor("bf", [1, 1], F32, kind="ExternalInput")  # 0.5*b_f
    out_d = nc.dram_tensor("out", [1, BL], F32, kind="ExternalOutput")

    with tile.TileContext(nc) as tc:
        with (
            tc.tile_pool(name="big", bufs=1) as big,
            tc.tile_pool(name="wpool", bufs=1) as wpool,
            tc.tile_pool(name="state", bufs=1) as state,
            tc.tile_pool(name="hpool", bufs=2) as hpool,
            tc.tile_pool(name="work", bufs=2) as work,
            tc.tile_pool(name="ps_sc", bufs=1, space="PSUM") as ps_sc_pool,
            tc.tile_pool(name="ps_rz", bufs=1, space="PSUM") as ps_rz_pool,
            tc.tile_pool(name="ps_hn", bufs=1, space="PSUM") as ps_hn_pool,
            tc.tile_pool(name="ps_in", bufs=1, space="PSUM") as ps_in_pool,
            tc.tile_pool(name="ps_sum", bufs=1, space="PSUM") as ps_sum_pool,
            tc.tile_pool(name="ps_bc", bufs=1, space="PSUM") as ps_bc_pool,
            tc.tile_pool(name="ps_ts", bufs=1, space="PSUM") as ps_ts_pool,
            tc.tile_pool(name="ps_ebc", bufs=1, space="PSUM") as ps_ebc_pool,
        ):
            # ACT table warmup: first activation triggers the walrus-inserted
            # table load (needs its own syncs) — keep it dependency-light.
            warm = state.tile([1, 2], F32, tag="warm")
            nc.gpsimd.memset(warm[:], 0.0)
            nc.scalar.activation(warm[:], warm[:], AF.Tanh)
            nc.scalar.activation(warm[:], warm[:], AF.Exp)

            # ---- load inputs into SBUF ----
            x_sb = big.tile([128, n_steps * BL], F32)
            nchunk = 8
            cw = (n_steps * BL) // nchunk
            for c in range(nchunk):
                nc.sync.dma_start(x_sb[:, c * cw:(c + 1) * cw], x_d[:, c * cw:(c + 1) * cw])

            def load_w(dram, shape, name):
                t = wpool.tile(shape, F32, tag=name)
                nc.sync.dma_start(t[:], dram[:])
                return t

            wa = load_w(wa_d, [128, 3 * 128], "wa")
            wih = load_w(wih_d, [128, 768], "wih")
            whh = load_w(whh_d, [128, 2 * 768], "whh")
            brz4 = load_w(brz4_d, [4, 128], "brz4")
            bin2 = load_w(bin2_d, [2, 128], "bin2")
            bhn2 = load_w(bhn2_d, [2, 128], "bhn2")
            sel4 = load_w(sel4_d, [4, 128], "sel4")
            sel2 = load_w(sel2_d, [2, 64], "sel2")
            ba = load_w(ba_d, [128, 1], "ba")
            wt = load_w(wt_d, [128, 2], "wt")
            bt = load_w(bt_d, [1, 1], "bt")
            wf = load_w(wf_d, [128, 2], "wf")
            bf = load_w(bf_d, [1, 1], "bf")

            x_mm = big.tile([128, n_steps * BL], MM_DT)
            for c in range(nchunk):
                nc.vector.tensor_copy(x_mm[:, c * cw:(c + 1) * cw],
                                      x_sb[:, c * cw:(c + 1) * cw])
            wa_m = wpool.tile([128, 3 * 128], MM_DT, tag="wa_m")
            nc.vector.tensor_copy(wa_m[:], wa[:])
            wih_m = wpool.tile([128, 768], MM_DT, tag="wih_m")
            nc.vector.tensor_copy(wih_m[:], wih[:])
            whh_m = wpool.tile([128, 2 * 768], MM_DT, tag="whh_m")
            nc.vector.tensor_copy(whh_m[:], whh[:])
            wt_m = wpool.tile([128, 2], MM_DT, tag="wt_m")
            nc.vector.tensor_copy(wt_m[:], wt[:])
            def to_mm(t, shape, name):
                tm = wpool.tile(shape, MM_DT, tag=name)
                nc.vector.tensor_copy(tm[:], t[:])
                return tm
            brz4_m = to_mm(brz4, [4, 128], "brz4_m")
            bin2_m = to_mm(bin2, [2, 128], "bin2_m")
            bhn2_m = to_mm(bhn2, [2, 128], "bhn2_m")
            sel4_m = to_mm(sel4, [4, 128], "sel4_m")
            sel2_m = to_mm(sel2, [2, 64], "sel2_m")

            ones_col = state.tile([128, 1], MM_DT, tag="ones_col")
            nc.vector.memset(ones_col[:], 1.0)
            ones_row = state.tile([1, 128], MM_DT, tag="ones_row")
            nc.vector.memset(ones_row[:], 1.0)
            ones_row_f = state.tile([1, 128], F32, tag="ones_row_f")
            nc.vector.memset(ones_row_f[:], 1.0)

            # ---- persistent state ----
            U = state.tile([128, 2 * BL], F32, tag="U")
            nc.vector.memset(U[:], 0.0)
            Z = state.tile([1, BL], F32, tag="Z")
            nc.vector.memset(Z[:], 0.0)
            h0_t = hpool.tile([128, 2 * BL], F32, tag="h")
            nc.vector.memset(h0_t[:], 0.0)
            hm0 = hpool.tile([128, 2 * BL], MM_DT, tag="hm")
            nc.vector.memset(hm0[:], 0.0)

            # ---- persistent PSUM tiles (8 banks total) ----
            ps_sc = ps_sc_pool.tile([128, BL], F32)
            ps_rz = ps_rz_pool.tile([128, 4 * BL], F32)
            ps_hn = ps_hn_pool.tile([128, 2 * BL], F32)
            ps_in = ps_in_pool.tile([128, 2 * BL], F32)
            ps_sum = ps_sum_pool.tile([1, BL], F32)
            ps_bc = ps_bc_pool.tile([128, BL], F32)
            ps_ts = ps_ts_pool.tile([1, BL], F32)
            ps_ebc = ps_ebc_pool.tile([128, 2 * BL], F32)

            h = h0_t   # fp32 h (state after previous step)
            hm = hm0   # matmul-dtype h

            MM = nc.tensor.matmul
            ones32 = ones_row[:, 0:BL]

            for t in range(n_steps):
                xt = x_mm[:, t * BL:(t + 1) * BL]

                # -- PE: scores = W_a @ [x_t; h]  (feature-major out [128, BL])
                MM(ps_sc[:], wa_m[:, 0:128], xt, start=True, stop=False)
                MM(ps_sc[:], wa_m[:, 128:256], hm[:, 0:BL], start=False, stop=False)
                MM(ps_sc[:], wa_m[:, 256:384], hm[:, BL:2 * BL], start=False, stop=True)

                # -- U-path part 1: t_score MMs on h (= enc state s_{t-1})
                if t > 0:
                    MM(ps_ts[:], wt_m[:, 0:1], hm[:, 0:BL], start=True, stop=False)
                    MM(ps_ts[:], wt_m[:, 1:2], hm[:, BL:2 * BL], start=False, stop=True)

                # -- PE: bias injections (no data deps — fill PE idle time)
                MM(ps_rz[:], brz4_m[:], sel4_m[:], start=True, stop=False)
                MM(ps_in[:], bin2_m[:], sel2_m[:], start=True, stop=False)
                MM(ps_hn[:], bhn2_m[:], sel2_m[:], start=True, stop=False)

                # -- PE: gh for r,z (accumulate; gi joins later) and for n
                def gh_rz(m, k):
                    MM(ps_rz[:, m * BL:(m + 1) * BL],
                       whh_m[:, k * 768 + m * 128:k * 768 + (m + 1) * 128],
                       hm[:, k * BL:(k + 1) * BL],
                       start=False, stop=False)

                def gh_hn(m, k):
                    MM(ps_hn[:, m * BL:(m + 1) * BL],
                       whh_m[:, k * 768 + (4 + m) * 128:k * 768 + (5 + m) * 128],
                       hm[:, k * BL:(k + 1) * BL],
                       start=False, stop=(k == 1))

                for m in range(3):
                    for k in range(2):
                        gh_rz(m, k)

                # -- ACT: exp(tanh(scores + b_a)); tanh in-place in PSUM so the
                # exp reads the cheaper PSUM port (172 vs 224 fixed cycles)
                nc.scalar.activation(ps_sc[:], ps_sc[:], AF.Tanh, bias=ba[:, 0:1])
                exp_sb = work.tile([128, BL], MM_DT, tag="exp_sb")
                exp_inst = nc.scalar.activation(exp_sb[:], ps_sc[:], AF.Exp)

                # -- U-path: e = exp(tanh(ts + b_t)); keep it BEHIND exp_s on
                # the ACT queue (the heap otherwise reorders it onto the chain)
                if t > 0:
                    ts_sb = work.tile([1, BL], F32, tag="ts_sb")
                    ts_inst = nc.scalar.activation(ts_sb[:], ps_ts[:], AF.Tanh,
                                                   bias=bt[:, 0:1])
                    add_dep_helper(ts_inst.ins, exp_inst.ins, sync=False,
                                   reason="ts pair after chain exp")
                    e_f32 = work.tile([1, BL], F32, tag="e_f32")
                    last_act = nc.scalar.activation(e_f32[:], ts_sb[:], AF.Exp)
                else:
                    e_f32 = None
                    last_act = exp_inst

                # -- PE: softmax denominator early (PE SEQ queue position),
                # then the rest of gh, then the 1/sum broadcast
                MM(ps_sum[:], ones_col[:], exp_sb[:], start=True, stop=True)
                rsum = work.tile([1, BL], F32, tag="rsum")
                nc.vector.reciprocal(rsum[:], ps_sum[:])
                for k in range(2):
                    gh_rz(3, k)
                for m in range(2):
                    for k in range(2):
                        gh_hn(m, k)
                MM(ps_bc[:], ones_row_f[:], rsum[:], start=True, stop=True)

                # evict hn to bf16 SBUF on the idle ACT (off-chain) for 2x nm;
                # pin it behind the chain ACT ops so the heap can't interleave it
                hn_sb = work.tile([128, 2 * BL], BF16, tag="hn_sb")
                ev_inst = nc.scalar.activation(hn_sb[:], ps_hn[:], AF.Copy)
                add_dep_helper(ev_inst.ins, last_act.ins, sync=False,
                               reason="hn evict after chain ACT ops")

                # -- VEC: xw = exp_s * x_t * bc   (softmax(scores) * x_t)
                y_sb = work.tile([128, BL], MM_DT, tag="y_sb")
                nc.vector.tensor_tensor(y_sb[:], exp_sb[:], xt, op=ALU.mult)
                xw = work.tile([128, BL], MM_DT, tag="xw")
                nc.vector.tensor_tensor(xw[:], y_sb[:], ps_bc[:], op=ALU.mult)

                # -- PE: gi joins the gate accumulations
                for m in range(4):
                    MM(ps_rz[:, m * BL:(m + 1) * BL],
                       wih_m[:, m * 128:(m + 1) * 128], xw[:],
                       start=False, stop=True)
                for m in range(2):
                    MM(ps_in[:, m * BL:(m + 1) * BL],
                       wih_m[:, (4 + m) * 128:(5 + m) * 128], xw[:],
                       start=False, stop=True)

                # -- U-path: broadcast e over partitions (slack)
                if e_f32 is not None:
                    MM(ps_ebc[:, 0:BL], ones_row_f[:], e_f32[:], start=True, stop=True)
                    MM(ps_ebc[:, BL:2 * BL], ones_row_f[:], e_f32[:], start=True, stop=True)

                # -- VEC: U/Z accumulation for current h (enc state s_{t-1})
                if e_f32 is not None:
                    tmp = work.tile([128, 2 * BL], F32, tag="tmp")
                    nc.vector.tensor_tensor(tmp[:], ps_ebc[:], h[:], op=ALU.mult)
                    nc.vector.tensor_tensor(U[:], U[:], tmp[:], op=ALU.add)
                    nc.vector.tensor_tensor(Z[:], Z[:], e_f32[:], op=ALU.add)

                # -- gates: tanh(0.5*(g+b)); sigma(x)=0.5+0.5*tanh(x/2) folded in.
                # r-half first (gates the n-path), z-half runs under the n-path.
                srz = work.tile([128, 4 * BL], BF16, tag="srz")
                nc.scalar.activation(srz[:, 0:2 * BL], ps_rz[:, 0:2 * BL],
                                     AF.Tanh, scale=0.5)
                nc.scalar.activation(srz[:, 2 * BL:4 * BL], ps_rz[:, 2 * BL:4 * BL],
                                     AF.Tanh, scale=0.5)
                tr = srz[:, 0:2 * BL]
                tz = srz[:, 2 * BL:4 * BL]

                # -- n = tanh(i_n + r*h_n);  r*h_n = (1+tr) * hn  (whh_n pre-halved)
                nm = work.tile([128, 2 * BL], BF16, tag="nm")
                nc.vector.scalar_tensor_tensor(nm[:], tr, 1.0, hn_sb[:],
                                               op0=ALU.add, op1=ALU.mult)
                npre = ps_ebc  # reuse: ebc consumed by the U-path already
                nc.vector.tensor_tensor(npre[:], nm[:], ps_in[:], op=ALU.add)
                # z-path precomputes (slack, overlap tanh_n): q=1-z, p2=(1+tz)*h
                q_sb = work.tile([128, 2 * BL], BF16, tag="q_sb")
                nc.vector.tensor_scalar(q_sb[:], tz, -0.5, 0.5, ALU.mult, ALU.add)
                p2 = work.tile([128, 2 * BL], BF16, tag="p2")
                nc.vector.scalar_tensor_tensor(p2[:], tz, 1.0, h[:],
                                               op0=ALU.add, op1=ALU.mult)
                n_sb = work.tile([128, 2 * BL], BF16, tag="n_sb")
                nc.scalar.activation(n_sb[:], npre[:], AF.Tanh)

                # -- h_new = (1-z)*n + z*h = q*n + 0.5*p2
                qn = work.tile([128, 2 * BL], BF16, tag="qn")
                nc.vector.tensor_tensor(qn[:], q_sb[:], n_sb[:], op=ALU.mult)
                hm_new = hpool.tile([128, 2 * BL], MM_DT, tag="hm")
                nc.vector.scalar_tensor_tensor(hm_new[:], p2[:], 0.5, qn[:],
                                               op0=ALU.mult, op1=ALU.add)
                h_new = hpool.tile([128, 2 * BL], F32, tag="h")
                nc.vector.scalar_tensor_tensor(h_new[:], p2[:], 0.5, qn[:],
                                               op0=ALU.mult, op1=ALU.add)

                h, hm = h_new, hm_new

            # ---- tail: U-path for the last h ----
            MM(ps_ts[:], wt_m[:, 0:1], hm[:, 0:BL], start=True, stop=False)
            MM(ps_ts[:], wt_m[:, 1:2], hm[:, BL:2 * BL], start=False, stop=True)
            ts_sb = work.tile([1, BL], F32, tag="ts_sb")
            nc.scalar.activation(ts_sb[:], ps_ts[:], AF.Tanh, bias=bt[:, 0:1])
            e_f32 = work.tile([1, BL], F32, tag="e_f32")
            nc.scalar.activation(e_f32[:], ts_sb[:], AF.Exp)
            MM(ps_ebc[:, 0:BL], ones_row_f[:], e_f32[:], start=True, stop=True)
            MM(ps_ebc[:, BL:2 * BL], ones_row_f[:], e_f32[:], start=True, stop=True)
            tmp = work.tile([128, 2 * BL], F32, tag="tmp")
            nc.vector.tensor_tensor(tmp[:], ps_ebc[:], h[:], op=ALU.mult)
            nc.vector.tensor_tensor(U[:], U[:], tmp[:], op=ALU.add)
            nc.vector.tensor_tensor(Z[:], Z[:], e_f32[:], op=ALU.add)

            # ---- context = U / Z ; logits = sigmoid(W_f @ context + b_f) ----
            rZ = work.tile([1, BL], F32, tag="rZ")
            nc.vector.reciprocal(rZ[:], Z[:])
            MM(ps_ebc[:, 0:BL], ones_row_f[:], rZ[:], start=True, stop=True)
            MM(ps_ebc[:, BL:2 * BL], ones_row_f[:], rZ[:], start=True, stop=True)
            ctx = work.tile([128, 2 * BL], MM_DT, tag="ctx")
            nc.vector.tensor_tensor(ctx[:], U[:], ps_ebc[:], op=ALU.mult)
            wf_m = wpool.tile([128, 2], MM_DT, tag="wf_m")
            nc.vector.tensor_copy(wf_m[:], wf[:])
            MM(ps_ts[:], wf_m[:, 0:1], ctx[:, 0:BL], start=True, stop=False)
            MM(ps_ts[:], wf_m[:, 1:2], ctx[:, BL:2 * BL], start=False, stop=True)
            sig_t = work.tile([1, BL], F32, tag="sig_t")
            nc.scalar.activation(sig_t[:], ps_ts[:], AF.Tanh, bias=bf[:, 0:1], scale=0.5)
            out_sb = work.tile([1, BL], F32, tag="out_sb")
            nc.vector.tensor_scalar(out_sb[:], sig_t[:], 0.5, 0.5, ALU.mult, ALU.add)
            nc.sync.dma_start(out_d[:], out_sb[:])

    return nc


_PROGRAM_CACHE = {}


def _get_program(n_steps: int):
    if n_steps not in _PROGRAM_CACHE:
        nc = _build_program(n_steps)
        nc.finalize()
        _PROGRAM_CACHE[n_steps] = nc
    return _PROGRAM_CACHE[n_steps]


def _prep_weights(W_a, b_a, W_ih, b_ih, W_hh, b_hh, W_t, b_t, W_f, b_f):
    f = np.float32
    wa = np.concatenate([W_a[:, 0:128].T, W_a[:, 128:256].T, W_a[:, 256:384].T],
                        axis=1).astype(f)                       # [128, 384]
    wih = np.ascontiguousarray(W_ih.T).astype(f)                # [128, 768]
    W_hh_s = W_hh.copy()
    W_hh_s[512:768, :] = 0.5 * W_hh_s[512:768, :]   # pre-halve n-gate h-side
    whh = np.concatenate([W_hh_s[:, 0:128].T, W_hh_s[:, 128:256].T], axis=1).astype(f)
    brz4 = np.asarray((b_ih + b_hh)[0:512]).reshape(4, 128).astype(f)
    bin2 = np.asarray(b_ih[512:768]).reshape(2, 128).astype(f)
    bhn2 = (0.5 * np.asarray(b_hh[512:768])).reshape(2, 128).astype(f)
    sel4 = np.zeros((4, 128), f)
    for m in range(4):
        sel4[m, m * 32:(m + 1) * 32] = 1.0
    sel2 = np.zeros((2, 64), f)
    for m in range(2):
        sel2[m, m * 32:(m + 1) * 32] = 1.0
    ba = np.asarray(b_a).reshape(128, 1).astype(f)
    wt = W_t.reshape(256).reshape(2, 128).T.astype(f)           # [128, 2]
    bt = np.array([[float(np.asarray(b_t).reshape(()))]], dtype=f)
    wf = W_f.reshape(256).reshape(2, 128).T.astype(f)
    bf = np.array([[0.5 * float(np.asarray(b_f).reshape(()))]], dtype=f)
    return dict(wa=wa, wih=wih, whh=whh, brz4=brz4, bin2=bin2, bhn2=bhn2,
                sel4=sel4, sel2=sel2, ba=ba, wt=wt, bt=bt, wf=wf, bf=bf)


def kernel(x, W_a, b_a, W_ih, b_ih, W_hh, b_hh, W_t, b_t, W_f, b_f,
           n_steps=None, trace=False):
    x = np.asarray(x, dtype=np.float32)
    n_steps = x.shape[1] if n_steps is None else n_steps
    nc = _get_program(n_steps)
    wmap = _prep_weights(np.asarray(W_a), np.asarray(b_a), np.asarray(W_ih),
                         np.asarray(b_ih), np.asarray(W_hh), np.asarray(b_hh),
                         np.asarray(W_t), np.asarray(b_t), np.asarray(W_f),
                         np.asarray(b_f))
    in_maps = []
    nb = x.shape[0] // NCORES
    for c in range(NCORES):
        xc = x[c * nb:(c + 1) * nb]                              # [nb, S, I]
        xf = np.ascontiguousarray(xc.transpose(2, 1, 0)).reshape(128, -1)
        m = dict(wmap)
        m["x"] = np.ascontiguousarray(xf, dtype=np.float32)
        in_maps.append(m)
    res = run_bass_kernel_spmd(nc, in_maps, core_ids=list(range(NCORES)),
                               trace=trace)
    out = np.zeros((x.shape[0], 1), dtype=np.float32)
    for c in range(NCORES):
        out[c * nb:(c + 1) * nb, 0] = res.results[c]["out"][0, :]
    kernel.last_results = res
    return out
